# revision 122
# baseline (speedup 1.0000x reference)
"""Trainium2 Bass kernel for MultiHeadSelfAttention with ALiBi + adj bias.

Reference computation (B=2, L=2048, H=1024, NH=16, HS=64):
    xp = x @ weights + in_bias                  # [b, l, 3h], per-head interleaved qkv
    q, k, v per head; att = q k^T / 8 + alibi + gamma*adj; softmax
    out = (att @ v) @ out_w + out_bias

Sharding: 8 cores = 2 batches x 4 slot-groups. Core (b, g) owns four head
"slots": plain0=8+2g, plain1=9+2g (no ALiBi), heavy=4+g (small slopes,
full attention), light=g (large slopes 2^-1..2^-4). Light-slot attention
is banded: weights with slope*|i-j| > ~24 are < e^-24 of the row max, so
only an 8-j-tile window around the diagonal is processed.

TimelineSim cost facts this design exploits: matmul costs out_free_cols
x 1 cyc at bf16, 0.5 at fp8e4 DoubleRow; ACT exp costs 0.83ns/col +
~185ns/instr, DVE ops 1.04ns/col; engines execute IN ORDER per engine,
so overlap comes from emission interleaving; DMA runs 360GB/s only when
the contiguous run is >=512B (180 below), with a serial 625ns HWDGE
descriptor-gen stage per dma_start.

Key structure (147.0us vs the 203.6us ACT-serial baseline):
  - QKV projection: fp8 DoubleRow 3-product (xh@wh + xh@wl + xl@wh),
    weights prescaled x16. x is [P,KP,2,NC,IC] so chunk DMAs hit the
    512B fast path; V-pairs and Q/K projections interleave per chunk so
    QK fills V's DMA-feed stalls. K lo-residual via one DVE
    scalar_tensor_tensor (no -16I matmul round-trip).
  - Phase B per chunk: two pipelined segments. seg1 = plain0 (ACT fp8
    exp) || heavy in 1-bank [P,2,256] half-tiles (DVE int16-Schraudolph
    exp: bf16 bits = trunc(A16*S + advp), where advp = int16 host
    tensor A16*(gamma*adj + slope*dist) + B16 fused into the exp via
    scalar_tensor_tensor - no adj/dist identity matmuls for heavy).
    seg2 = plain1 + light (ACT) || prev-chunk alibi transpose + output
    projection pairs (DVE evacs). att@V consumes SBUF E tiles one unit
    behind so PE never stalls on exp latency.
  - PSUM (8 banks exactly): sp 2x[P,2,IC] (4) + psX 2x1-bank (2) +
    atp 1 (p0 then p1) + atf 1 (hv then lt).
  - plain/light: gamma*adj (+ slope*dist for light) still ride fp8
    hi/lo DoubleRow identity matmuls (ACT exp can't fuse a tensor add;
    DVE fp8-Schraudolph fails the error budget - measured).
  - Output yT is [P, 4, 2, L] so each outproj pair is ONE dma_start
    (HWDGE serial stage was the tail bottleneck); host de-permutes.
  - Epilogue: last chunk's outproj as [P,4,256] quads with ACT/DVE
    alternating evacs. Light slot: 3 units' S+exp in seg1 (ACT slack),
    1 in seg2, att@V all in seg2 (atf bank time-share); dist ships as
    gathered 8-tile windows [P,NC,2,LWIN,2,256] (half the bytes).
    DMA from PSUM is ILLEGAL (dma_start asserts SBUF/DRAM source).
Host sums the 4 yT partials per batch, de-permutes, adds out_bias.
Error budget is tight: 1.21e-2 of the 2e-2 gate. Verified-failing
cheaper variants: fp8 outproj (2.6e-2), V 2-product (2.5e-2), Q
2-product (2.9e-2), light-slot Schraudolph (1.7e-2 + slower).
"""

import numpy as np
import ml_dtypes
from contextlib import ExitStack

import concourse.tile as tile
from concourse import bacc, mybir
from concourse import bass_utils

F32 = mybir.dt.float32
F32R = mybir.dt.float32r
BF16 = mybir.dt.bfloat16
F8 = mybir.dt.float8e4
AF = mybir.ActivationFunctionType
DR = mybir.MatmulPerfMode.DoubleRow
NPF8 = ml_dtypes.float8_e4m3
NPBF16 = ml_dtypes.bfloat16

B, L, H, NH = 2, 2048, 1024, 16
HS = 64
P = 128            # partition tile
IC = 512           # i-chunk width
NC = L // IC       # 4 i-chunks
NJ = L // P        # 16 j tiles
KT = H // P        # 8 contraction tiles over hidden
KP = KT // 2       # 4 k-tile pairs (DoubleRow planes)
SCALE = 0.125      # 1/sqrt(HS)
WS = 16.0          # weight prescale (fp8 subnormal clearance)
E8SHIFT = -4.0     # exp shift keeps fp8 E8 = exp(S-4) under e4m3 max 240
LWIN = 8           # light-slot j-tile window per 256-col sub-chunk

RUN_KWARGS: dict = {}
_cache: dict = {}

# Schraudolph fast-exp constants for DVE int16-bitcast bf16 exp:
# bf16 bits = trunc(A16*S + B16); bitcast(bits) ~ exp(S). Valid for
# S > -88 (bits stay positive); alibi slots satisfy this by windowing
# (light) or slope <= 1/32 (heavy).
A16 = 128.0 / float(np.log(2.0))
B16 = 127.0 * 128.0 + 0.5


def _build_program(with_qk_bias=False):
    nc = bacc.Bacc("TRN2", target_bir_lowering=False, debug=False,
                   enable_asserts=False, num_devices=8)

    xh = nc.dram_tensor("xh", [P, KP, 2, NC, IC], F8,
                        kind="ExternalInput").ap()
    xl = nc.dram_tensor("xl", [P, KP, 2, NC, IC], F8,
                        kind="ExternalInput").ap()
    wq = nc.dram_tensor("wq", [P, KP, 2, 2, 512], F8,
                        kind="ExternalInput").ap()
    wv = nc.dram_tensor("wv", [P, KP, 2, 2, 260], F8,
                        kind="ExternalInput").ap()
    # cst = [ones(512) | vb(260)] in one tensor/DMA
    cst = nc.dram_tensor("cst", [1, IC + 260], BF16,
                         kind="ExternalInput").ap()
    qkb = (nc.dram_tensor("qkb", [1, 512], BF16, kind="ExternalInput").ap()
           if with_qk_bias else None)
    adjp = nc.dram_tensor("adjp", [P, NJ, 2, L], F8, kind="ExternalInput").ap()
    # light-slot dist hi/lo planes gathered to the 8-j-tile windows:
    # [p, chunk, sub, j-in-window, plane, col] (plane-col contiguous)
    distw = nc.dram_tensor("distw", [P, NC, 2, LWIN, 2, 256], F8,
                           kind="ExternalInput").ap()
    # heavy slot's A16*(gamma*adj + slope*dist) + B16, int16, per chunk
    advph = nc.dram_tensor("advph", [P, NJ, NC, 2, 256], mybir.dt.int16,
                           kind="ExternalInput").ap()
    idg = nc.dram_tensor("idg", [P, 2, 512], F8, kind="ExternalInput").ap()
    idsl = nc.dram_tensor("idsl", [P, 2, 256], F8, kind="ExternalInput").ap()
    identT = nc.dram_tensor("identT", [P, P], BF16, kind="ExternalInput").ap()
    ow = nc.dram_tensor("ow", [P, 2, H], BF16, kind="ExternalInput").ap()
    # output y^T stored [p, m-pair, m-in-pair, l]; host de-permutes to [H, L]
    yT = nc.dram_tensor("yT", [P, 4, 2, L], BF16, kind="ExternalOutput").ap()
    DEBUG = bool(_cache.get("debug"))
    if DEBUG:
        dbg_attnT0 = nc.dram_tensor("dbg_attnT0", [P, L], BF16, kind="ExternalOutput").ap()
        dbg_attnT1 = nc.dram_tensor("dbg_attnT1", [P, L], BF16, kind="ExternalOutput").ap()
        dbg_q80 = nc.dram_tensor("dbg_q80", [P, L], F8, kind="ExternalOutput").ap()
        dbg_q81 = nc.dram_tensor("dbg_q81", [P, L], F8, kind="ExternalOutput").ap()
        dbg_khl0 = nc.dram_tensor("dbg_khl0", [P, 2, L], F8, kind="ExternalOutput").ap()
        dbg_khl1 = nc.dram_tensor("dbg_khl1", [P, 2, L], F8, kind="ExternalOutput").ap()
        dbg_v0 = nc.dram_tensor("dbg_v0", [P, 260], BF16, kind="ExternalOutput").ap()
        dbg_v800 = nc.dram_tensor("dbg_v800", [P, 2, P], F8, kind="ExternalOutput").ap()
        dbg_atfh = nc.dram_tensor("dbg_atfh", [P, 4, 65], F32, kind="ExternalOutput").ap()
        dbg_attf0 = nc.dram_tensor("dbg_attf0", [P, 4, HS], BF16, kind="ExternalOutput").ap()
        dbg_et0 = nc.dram_tensor("dbg_et0", [P, 2, IC], BF16, kind="ExternalOutput").ap()

    with tile.TileContext(nc) as tc, ExitStack() as ctx:
        # ---- long-lived tensors ----
        persist = ctx.enter_context(tc.tile_pool(name="persist", bufs=1))
        # Q col-major fp8 (1/8 folded): tile0 rows = Q(p0 0:64, p1 64:128),
        # tile1 = Q(hv, lt). K as (hi, lo) fp8 plane pairs, same row split.
        q8_sb = [persist.tile([P, L], F8, tag=f"q8{m}", name=f"q8{m}")
                 for m in range(2)]
        khl_sb = [persist.tile([P, 2, L], F8, tag=f"khl{m}", name=f"khl{m}")
                  for m in range(2)]
        # V token-major bf16 [tok, 4*65] (65th col of each slot = 1.0)
        v_sb = [persist.tile([P, 260], BF16, tag=f"v{t}", name=f"v{t}")
                for t in range(NJ)]
        # V fp8 j-pair tiles for plain-slot DoubleRow att@V. Walrus requires
        # DoubleRow stationary planes of exactly 128 columns, so each slot's
        # 65 V columns (65th = ones) are zero-padded to 128.
        v8_sb = [[persist.tile([P, 2, P], F8, tag=f"v8_{s}_{t}",
                               name=f"v8_{s}_{t}") for t in range(NJ // 2)]
                 for s in range(2)]
        # normalized attention, d-major: tile0 = plain0+plain1, tile1 = hv+lt
        attnT = [persist.tile([P, L], BF16, tag=f"attnT{m}", name=f"attnT{m}")
                 for m in range(2)]
        # alibi slots' flipped attention [i, d], double-buffered by chunk
        # parity (transpose to d-major happens one chunk later)
        attn_f = [[persist.tile([P, 4, HS], BF16, tag=f"attf{par}_{m}",
                                name=f"attf{par}_{m}") for m in range(2)]
                  for par in range(2)]
        cst_sb = persist.tile([1, IC + 260], BF16, tag="cst")
        nc.sync.dma_start(cst_sb[:], cst)
        # B-phase-only tiles: allocated here, DMA'd after the x stream so
        # their HWDGE slots don't delay phase A's first matmuls
        idg_sb = persist.tile([P, 2, 512], F8, tag="idg")
        idsl_sb = persist.tile([P, 2, 256], F8, tag="idsl")
        idT_sb = persist.tile([P, P], BF16, tag="identT")
        ow_sb = persist.tile([P, 2, H], BF16, tag="ow")
        if with_qk_bias:
            qkb_sb = persist.tile([1, 512], BF16, tag="qkb")
            nc.sync.dma_start(qkb_sb[:], qkb)
        mb_sb = persist.tile([P, 1], F32, tag="mb")
        nc.vector.memset(mb_sb[:], E8SHIFT)
        for s in range(2):
            for t in range(NJ // 2):
                nc.vector.memset(v8_sb[s][t][:], 0.0)

        # ========== Phase A: QKV projection (fp8 3-product) ==========
        bc_pool = ctx.enter_context(tc.tile_pool(name="bias_ch", bufs=3))
        adj_pre = {}
        dist_pre = {}
        advp_pre = {}
        with tc.tile_pool(name="xw", bufs=1) as xw_pool, \
             tc.tile_pool(name="psA", bufs=1, space="PSUM") as psA:
            wv_sb = xw_pool.tile([P, KP, 2, 2, 260], F8, tag="wv")
            nc.sync.dma_start(wv_sb[:], wv)
            xh_sb = xw_pool.tile([P, KP, 2, NC, IC], F8, tag="xh")
            xl_sb = xw_pool.tile([P, KP, 2, NC, IC], F8, tag="xl")
            nc.sync.dma_start(xh_sb[:, :, :, 0, :], xh[:, :, :, 0, :])
            nc.sync.dma_start(xl_sb[:, :, :, 0, :], xl[:, :, :, 0, :])
            wq_sb = xw_pool.tile([P, KP, 2, 2, 512], F8, tag="wq")
            nc.sync.dma_start(wq_sb[:], wq)
            for cc in range(1, NC):
                nc.sync.dma_start(xh_sb[:, :, :, cc, :], xh[:, :, :, cc, :])
                nc.sync.dma_start(xl_sb[:, :, :, cc, :], xl[:, :, :, cc, :])
            nc.sync.dma_start(idg_sb[:], idg)
            nc.sync.dma_start(idsl_sb[:], idsl)
            nc.sync.dma_start(idT_sb[:], identT)
            nc.sync.dma_start(ow_sb[:], ow)
            for c0 in range(1):
                csl = slice(c0 * IC, (c0 + 1) * IC)
                a_t = bc_pool.tile([P, NJ, 2, IC], F8, tag="adj",
                                   name=f"adj_pre{c0}", bufs=2)
                nc.sync.dma_start(a_t[:], adjp[:, :, :, csl])
                adj_pre[c0] = a_t
                d_t = bc_pool.tile([P, 2, LWIN, 2, 256], F8, tag="dist",
                                   name=f"dist_pre{c0}", bufs=2)
                nc.sync.dma_start(d_t[:], distw[:, c0, :, :, :, :])
                dist_pre[c0] = d_t
                h_t = bc_pool.tile([P, NJ, 2, 256], mybir.dt.int16,
                                   tag="advp", name=f"advp_pre{c0}", bufs=2)
                nc.sync.dma_start(h_t[:], advph[:, :, c0, :, :])
                advp_pre[c0] = h_t

            # V token-major (psum = 16*v; evacuations scale by 1/16) and
            # Q^T/K^T col-major, interleaved by 512-token chunk so QK
            # compute fills the V pipeline's DMA-feed stalls.
            def v_pair(tp):
                ps = psA.tile([P, 2, 512], F32, tag="vp", bufs=2)
                for tt in range(2):
                    t = 2 * tp + tt
                    tcc, tin = divmod(t, 4)
                    tsl = slice(tin * P, (tin + 1) * P)
                    if with_qk_bias:
                        nc.tensor.matmul(ps[:, tt, 0:260], cst_sb[:, 0:P],
                                         cst_sb[:, IC:IC + 260], start=True,
                                         stop=False, skip_group_check=True)
                    else:
                        # in_bias is zero: only the denominator ones
                        # columns (64, 129, 194, 259) need seeding
                        for sl4 in range(4):
                            oc = 65 * sl4 + 64
                            nc.tensor.matmul(
                                ps[:, tt, oc:oc + 1], cst_sb[:, 0:P],
                                cst_sb[:, IC + oc:IC + oc + 1],
                                start=(sl4 == 0), stop=False,
                                skip_group_check=True)
                    # pi-major so the xl-dependent products come last and
                    # PE starts on xh alone while xl still streams
                    for pi, (sta, hl) in enumerate(
                            ((xh_sb, 0), (xh_sb, 1), (xl_sb, 0))):
                        for kp in range(KP):
                            for hf in range(2):
                                nc.tensor.matmul(
                                    ps[:, tt, hf * 130:(hf + 1) * 130],
                                    sta[:, kp, :, tcc, tsl],
                                    wv_sb[:, kp, :, hl,
                                          hf * 130:(hf + 1) * 130],
                                    start=False,
                                    stop=(kp == KP - 1 and pi == 2
                                          and hf == 1 and tt == 1),
                                    perf_mode=DR, skip_group_check=True)
                for tt in range(2):
                    nc.vector.tensor_scalar_mul(v_sb[2 * tp + tt][:],
                                                ps[:, tt, 0:260], 1.0 / WS)
                for s in range(2):
                    nc.scalar.activation(
                        v8_sb[s][tp][:, :, 0:65],
                        ps[:, :, s * 65:(s + 1) * 65], AF.Copy,
                        scale=1.0 / WS)

            def qk_proj(m, cc):
                csl = slice(cc * IC, (cc + 1) * IC)
                ps = psA.tile([P, 2, 256], F32, tag="qkp", bufs=4)
                for half in range(2):
                    hsl = slice(half * 256, (half + 1) * 256)
                    if with_qk_bias:
                        nc.tensor.matmul(ps[:, half, :],
                                         qkb_sb[:, m * P:(m + 1) * P],
                                         cst_sb[:, 0:256],
                                         start=(half == 0), stop=False,
                                         skip_group_check=True)
                    prods = ((0, xh_sb), (1, xh_sb), (0, xl_sb))
                    for pi, (hl, mov) in enumerate(prods):
                        for kp in range(KP):
                            nc.tensor.matmul(
                                ps[:, half, :],
                                wq_sb[:, kp, :, hl, m * P:(m + 1) * P],
                                mov[:, kp, :, cc, hsl],
                                start=(half == 0 and kp == 0 and pi == 0
                                       and not with_qk_bias),
                                stop=(kp == KP - 1 and pi == 2
                                      and m < 2 and half == 1),
                                perf_mode=DR, skip_group_check=True)
                if m < 2:
                    nc.scalar.activation(q8_sb[m][:, csl], ps[:], AF.Copy,
                                         scale=SCALE / WS)
                else:
                    # K hi/lo split: hi = fp8(psum/WS) on ACT, then
                    # lo = psum/WS - hi in one DVE pass (replaces the old
                    # -WS*I follow-up matmul + second ACT evac)
                    kh = khl_sb[m - 2]
                    nc.scalar.activation(kh[:, 0, csl], ps[:], AF.Copy,
                                         scale=1.0 / WS)
                    nc.vector.scalar_tensor_tensor(
                        kh[:, 1, csl], ps[:], 1.0 / WS, kh[:, 0, csl],
                        op0=mybir.AluOpType.mult,
                        op1=mybir.AluOpType.subtract)

            for cc in range(NC):
                v_pair(2 * cc)
                v_pair(2 * cc + 1)
                for m in (1, 3, 0, 2):
                    qk_proj(m, cc)

        # ========== Phase B: two-segment pipelined attention ==========
        # Chunk = seg1 (p0-pipe on ACT ∥ hv half-tile-pipe on DVE) then
        # seg2 (p1+lt pipes on ACT ∥ prev-chunk transpose + outproj pairs
        # with DVE evacs). att@V consumes SBUF E tiles one unit behind so
        # PE never waits on exp latency. PSUM: sp_main 2x[P,2,IC] (4
        # banks), sp_aux 2x1-bank (hv halves / op pairs / transpose),
        # atp 1 bank (p0 then p1), atf 1 bank (hv then lt).
        with tc.tile_pool(name="e8_pool", bufs=8) as e8_pool, \
             tc.tile_pool(name="e_pool", bufs=8) as e_pool, \
             tc.tile_pool(name="r_pool", bufs=4) as r_pool, \
             tc.tile_pool(name="y_pool", bufs=6) as y_pool, \
             tc.tile_pool(name="psS", bufs=2, space="PSUM") as psS, \
             tc.tile_pool(name="psX", bufs=2, space="PSUM") as psX, \
             tc.tile_pool(name="psAc", bufs=1, space="PSUM") as psAc, \
             tc.tile_pool(name="psF", bufs=1, space="PSUM") as psF:

            def s_plain(s, c, jp, adj_ch):
                spp = psS.tile([P, 2, IC], F32, tag="sp", name=f"sp_p{s}")
                hp = slice(s * HS, (s + 1) * HS)
                for jj in range(2):
                    j = 2 * jp + jj
                    for h2 in range(2):
                        h2s = slice(h2 * 256, (h2 + 1) * 256)
                        qmv = q8_sb[0][hp, c * IC + h2 * 256:
                                       c * IC + (h2 + 1) * 256]
                        nc.tensor.matmul(
                            spp[:, jj, h2s],
                            khl_sb[0][hp, :, j * P:(j + 1) * P],
                            qmv.unsqueeze(1).broadcast_to([HS, 2, 256]),
                            start=True, stop=False, perf_mode=DR)
                        nc.tensor.matmul(
                            spp[:, jj, h2s], idg_sb[:, :, s * P:(s + 1) * P],
                            adj_ch[:, j, :, h2s], start=False, stop=True,
                            perf_mode=DR)
                e8p = e8_pool.tile([P, 2, IC], F8, tag="e8")
                nc.scalar.activation(e8p[:], spp[:], AF.Exp, bias=mb_sb[:])
                return e8p

            def av_plain(s, atp, e8p, jp):
                for h2 in range(2):
                    h2s = slice(h2 * 256, (h2 + 1) * 256)
                    nc.tensor.matmul(
                        atp[:, h2s], v8_sb[s][jp][:], e8p[:, :, h2s],
                        start=(jp == 0 and h2 == 0), stop=(jp == NJ // 2 - 1),
                        perf_mode=DR, skip_group_check=True)

            def s_heavy_half(c, j, advp_ch):
                """One j-tile of the heavy slot, full IC cols: [P,2,256]
                psum half-tiles in psX (1 bank). QK only on PE; the
                A16*(gamma*adj+slope*dist)+B16 bias rides the DVE
                Schraudolph exp as a fused int16 tensor add."""
                spp = psX.tile([P, 2, 256], F32, tag="px", name="sp_hv")
                for h2 in range(2):
                    qmv = q8_sb[1][0:HS, c * IC + h2 * 256:
                                   c * IC + (h2 + 1) * 256]
                    nc.tensor.matmul(
                        spp[:, h2, :],
                        khl_sb[1][0:HS, :, j * P:(j + 1) * P],
                        qmv.unsqueeze(1).broadcast_to([HS, 2, 256]),
                        start=True, stop=True, perf_mode=DR)
                et = e_pool.tile([P, 2, 256], BF16, tag="et")
                nc.vector.scalar_tensor_tensor(
                    et[:].bitcast(mybir.dt.int16), spp[:], A16,
                    advp_ch[:, j, :, :], op0=mybir.AluOpType.mult,
                    op1=mybir.AluOpType.add)
                return et

            def av_heavy(atf_h, et, j):
                # et[:, h2, :] covers i-cols h2*256 of the chunk; blocks of
                # 128 i map to atf rows via [i, d] flipped layout
                for h2 in range(2):
                    for b2 in range(2):
                        blk = 2 * h2 + b2
                        nc.tensor.matmul(
                            atf_h[:, blk, :],
                            et[:, h2, b2 * P:(b2 + 1) * P],
                            v_sb[j][:, 130:195],
                            start=(j == 0 and h2 == 0 and b2 == 0),
                            stop=(j == NJ - 1), skip_group_check=True)

            def s_light(c, k, adj_ch, dist_ch):
                sub, jq = divmod(k, 2)
                jw = min(max(4 * c + 2 * sub - 3, 0), NJ - LWIN)
                ssl = slice(c * IC + sub * 256, c * IC + sub * 256 + 256)
                bsl = slice(sub * 256, sub * 256 + 256)
                qmv = q8_sb[1][HS:P, ssl].unsqueeze(1)
                spq = psS.tile([P, 4, 256], F32, tag="sp", name="sp_lt")
                for j4 in range(4):
                    j = jw + 4 * jq + j4
                    nc.tensor.matmul(
                        spq[:, j4, :],
                        khl_sb[1][HS:P, :, j * P:(j + 1) * P],
                        qmv.broadcast_to([HS, 2, 256]),
                        start=True, stop=False, perf_mode=DR)
                    nc.tensor.matmul(spq[:, j4, :],
                                     idg_sb[:, :, 3 * P:4 * P],
                                     adj_ch[:, j, :, bsl],
                                     start=False, stop=False, perf_mode=DR)
                    nc.tensor.matmul(spq[:, j4, :],
                                     idsl_sb[:, :, P:2 * P],
                                     dist_ch[:, sub, 4 * jq + j4, :, :],
                                     start=False, stop=True, perf_mode=DR)
                et = e_pool.tile([P, 4, 256], BF16, tag="etl")
                nc.scalar.activation(et[:], spq[:], AF.Exp)
                return et

            def av_light(atf_l, et, c, k):
                sub, jq = divmod(k, 2)
                jw = min(max(4 * c + 2 * sub - 3, 0), NJ - LWIN)
                for j4 in range(4):
                    for blk in range(2):
                        nc.tensor.matmul(
                            atf_l[:, sub * 2 + blk, :],
                            et[:, j4, blk * P:(blk + 1) * P],
                            v_sb[jw + 4 * jq + j4][:, 195:260],
                            start=(sub == 0 and jq == 0 and j4 == 0
                                   and blk == 0),
                            stop=(sub == 1 and jq == 1 and j4 == 3),
                            skip_group_check=True)

            def transpose_prev_fi(pc, fi):
                rp = slice(fi * HS, (fi + 1) * HS)
                pcsl = slice(pc * IC, (pc + 1) * IC)
                pst = psX.tile([HS, 4, P], BF16, tag="px", name="pst")
                for k4 in range(4):
                    nc.tensor.matmul(pst[:, k4, :],
                                     attn_f[pc % 2][fi][:, k4, :],
                                     idT_sb[:], start=True, stop=True,
                                     is_transpose=True)
                nc.vector.tensor_copy(attnT[1][rp, pcsl], pst[:])

            def transpose_prev(pc):
                for fi in range(2):
                    transpose_prev_fi(pc, fi)

            def op_pair(pc, p, act_ok=False):
                hf, mp = divmod(p, 4)
                ysl = slice(pc * IC + hf * 256, pc * IC + (hf + 1) * 256)
                ps = psX.tile([P, 2, 256], F32, tag="px", name=f"op{p % 2}")
                for t in range(2):
                    m = 2 * mp + t
                    nc.tensor.matmul(ps[:, t, :],
                                     ow_sb[:, 0, m * P:(m + 1) * P],
                                     attnT[0][:, ysl],
                                     start=True, stop=False)
                    nc.tensor.matmul(ps[:, t, :],
                                     ow_sb[:, 1, m * P:(m + 1) * P],
                                     attnT[1][:, ysl],
                                     start=False, stop=True)
                yt = y_pool.tile([P, 2, 256], BF16, tag="yt")
                if act_ok and p % 2 == 0:
                    nc.scalar.activation(yt[:], ps[:], AF.Copy, scale=0.125)
                else:
                    nc.vector.tensor_scalar_mul(yt[:], ps[:], 0.125)
                nc.sync.dma_start(yT[:, mp, :, ysl], yt[:])

            def norm_plain(c, s, atp):
                csl = slice(c * IC, (c + 1) * IC)
                hp = slice(s * HS, (s + 1) * HS)
                rec = r_pool.tile([1, IC], F32R, tag="rec")
                with nc.allow_low_precision(reason="softmax recip"):
                    nc.vector.reciprocal(rec[:], atp[64:65, :])
                rbs = r_pool.tile([HS, IC], F32, tag="rbs")
                nc.gpsimd.partition_broadcast(rbs[:], rec[:].bitcast(F32))
                nc.vector.tensor_mul(attnT[0][hp, csl], atp[0:HS, :],
                                     rbs[:])

            def norm_flip(c, fi, atf):
                rec4 = r_pool.tile([P, 4], F32, tag="rec4")
                with nc.allow_low_precision(reason="softmax recip"):
                    nc.vector.reciprocal(rec4[:], atf[:, :, 64])
                for blk in range(4):
                    nc.vector.tensor_scalar_mul(
                        attn_f[c % 2][fi][:, blk, :],
                        atf[:, blk, 0:HS], rec4[:, blk:blk + 1])

            for c in range(NC):
                csl = slice(c * IC, (c + 1) * IC)
                nxt = c + 1
                if nxt < NC and nxt not in adj_pre:
                    nsl = slice(nxt * IC, (nxt + 1) * IC)
                    a_t = bc_pool.tile([P, NJ, 2, IC], F8, tag="adj",
                                       name=f"adj{nxt}", bufs=2)
                    nc.sync.dma_start(a_t[:], adjp[:, :, :, nsl])
                    adj_pre[nxt] = a_t
                    d_t = bc_pool.tile([P, 2, LWIN, 2, 256], F8, tag="dist",
                                       name=f"dist{nxt}", bufs=2)
                    nc.sync.dma_start(d_t[:], distw[:, nxt, :, :, :, :])
                    dist_pre[nxt] = d_t
                    h_t = bc_pool.tile([P, NJ, 2, 256], mybir.dt.int16,
                                       tag="advp", name=f"advp{nxt}", bufs=2)
                    nc.sync.dma_start(h_t[:], advph[:, :, nxt, :, :])
                    advp_pre[nxt] = h_t
                adj_ch = adj_pre[c]
                dist_ch = dist_pre[c]
                advp_ch = advp_pre[c]

                # ---- segment 1: p0 + lt S/exp (ACT) || hv halves (DVE);
                # lt's att@V is deferred to seg2 (atf bank time-share) ----
                atp = psAc.tile([P, IC], F32, tag="atp", name="atp0")
                atf_h = psF.tile([P, 4, 65], F32, tag="atf", name="atf_h")
                lt_ets = []
                pe8 = pet = None
                for jp in range(NJ // 2):
                    if jp == 0 and c > 0:
                        transpose_prev(c - 1)
                    et0 = s_heavy_half(c, 2 * jp, advp_ch)
                    e8 = s_plain(0, c, jp, adj_ch)
                    if pet is not None:
                        av_heavy(atf_h, pet[0], pet[1])
                    et1 = s_heavy_half(c, 2 * jp + 1, advp_ch)
                    av_heavy(atf_h, et0, 2 * jp)
                    if pe8 is not None:
                        av_plain(0, atp, pe8, jp - 1)
                    if jp % 2 == 1 and jp < 7:
                        k = jp // 2
                        lt_ets.append((s_light(c, k, adj_ch, dist_ch), k))
                    pe8, pet = e8, (et1, 2 * jp + 1)
                av_heavy(atf_h, pet[0], pet[1])
                av_plain(0, atp, pe8, NJ // 2 - 1)
                norm_plain(c, 0, atp)
                norm_flip(c, 0, atf_h)

                # ---- segment 2: p1 (ACT) || lt att@V + transpose +
                # outproj pairs (DVE evacs) ----
                atp = psAc.tile([P, IC], F32, tag="atp", name="atp1")
                atf_l = psF.tile([P, 4, 65], F32, tag="atf", name="atf_l")
                pe8 = None
                for jp in range(NJ // 2):
                    e8 = s_plain(1, c, jp, adj_ch)
                    if jp == 1:
                        lt_ets.append((s_light(c, 3, adj_ch, dist_ch), 3))
                    if pe8 is not None:
                        av_plain(1, atp, pe8, jp - 1)
                    if jp % 2 == 1:
                        lte, k = lt_ets[jp // 2]
                        av_light(atf_l, lte, c, k)
                    if c > 0:
                        op_pair(c - 1, jp)
                    pe8 = e8
                av_plain(1, atp, pe8, NJ // 2 - 1)
                norm_plain(c, 1, atp)
                norm_flip(c, 1, atf_l)

            # epilogue: last chunk's transpose + output projection. The
            # contraction splits at attnT row 64 so the p0/hv/lt halves
            # (ready early) run while p1's normalize chain drains; quads
            # ride the freed psS banks with wide ACT/DVE evacs.
            pc = NC - 1

            transpose_prev(pc)
            for q in range(4):
                hf, mq = divmod(q, 2)
                ysl = slice(pc * IC + hf * 256, pc * IC + (hf + 1) * 256)
                ps = psS.tile([P, 4, 256], F32, tag="sp", name=f"epi{q % 2}")
                for t in range(4):
                    m = 4 * mq + t
                    nc.tensor.matmul(
                        ps[:, t, :], ow_sb[:, 0, m * P:(m + 1) * P],
                        attnT[0][:, ysl],
                        start=(t % 2 == 0), stop=False,
                        skip_group_check=True)
                    nc.tensor.matmul(
                        ps[:, t, :], ow_sb[:, 1, m * P:(m + 1) * P],
                        attnT[1][:, ysl],
                        start=False, stop=True,
                        skip_group_check=True)
                yt = y_pool.tile([P, 4, 256], BF16, tag="ytq")
                if q % 2 == 0:
                    nc.scalar.activation(yt[:], ps[:], AF.Copy, scale=0.125)
                else:
                    nc.vector.tensor_scalar_mul(yt[:], ps[:], 0.125)
                for k, mp in enumerate((2 * mq, 2 * mq + 1)):
                    nc.sync.dma_start(yT[:, mp, :, ysl],
                                      yt[:, 2 * k:2 * k + 2, :])

        if DEBUG:
            nc.sync.dma_start(dbg_attnT0, attnT[0][:])
            nc.sync.dma_start(dbg_attnT1, attnT[1][:])
            nc.sync.dma_start(dbg_q80, q8_sb[0][:])
            nc.sync.dma_start(dbg_q81, q8_sb[1][:])
            nc.sync.dma_start(dbg_khl0, khl_sb[0][:])
            nc.sync.dma_start(dbg_khl1, khl_sb[1][:])
            nc.sync.dma_start(dbg_v0, v_sb[0][:])
            nc.sync.dma_start(dbg_v800, v8_sb[0][0][:])
    nc.compile()
    return nc


def _alibi_slopes():
    n = NH // 2
    start = 2.0 ** (-(2.0 ** (-(np.log2(n) - 3.0))))
    s = np.array([start * start ** i for i in range(n)], dtype=np.float32)
    return np.concatenate([s, np.zeros(n, dtype=np.float32)])


def _hl8(a):
    """Split array into (hi, lo) fp8 e4m3 pair; hi + lo ~ a to ~12 bits."""
    hi = a.astype(NPF8)
    lo = (a - hi.astype(np.float32)).astype(NPF8)
    return hi, lo


def _pairs_P_NJ(hi, lo):
    """[L, L] hi/lo -> [P, NJ, 2, L] fp8 (partition-major j tiles)."""
    out = np.empty((P, NJ, 2, L), dtype=NPF8)
    out[:, :, 0, :] = hi.reshape(NJ, P, L).transpose(1, 0, 2)
    out[:, :, 1, :] = lo.reshape(NJ, P, L).transpose(1, 0, 2)
    return out


def _kp_pairs(a, cols):
    """[H, cols] fp8 -> [P, KP, 2, cols] (k-tile pair planes)."""
    return np.ascontiguousarray(
        a.reshape(KP, 2, P, cols).transpose(2, 0, 1, 3))


def _build_in_maps(x, adj, weights, in_bias, gamma, out_w, with_qk_bias):
    from concurrent.futures import ThreadPoolExecutor
    slopes = _alibi_slopes()
    ar = np.arange(L, dtype=np.float32)
    dist = -np.abs(ar[None, :] - ar[:, None])

    # dist = 16*hi + lo EXACTLY (lo is an integer in [-8, 8]); gather the
    # light-slot windows: [P, NC, 2, LWIN, 2, 256]
    dhi = (dist / 16.0).astype(NPF8)
    dlo = (dist - 16.0 * dhi.astype(np.float32)).astype(NPF8)
    distw = np.empty((P, NC, 2, LWIN, 2, 256), NPF8)
    for c in range(NC):
        for sub in range(2):
            jw = min(max(4 * c + 2 * sub - 3, 0), NJ - LWIN)
            i0 = c * IC + sub * 256
            for pl, arr in ((0, dhi), (1, dlo)):
                blk = arr[jw * P:(jw + LWIN) * P, i0:i0 + 256]
                distw[:, c, sub, :, pl, :] = blk.reshape(
                    LWIN, P, 256).transpose(1, 0, 2)

    def _adjp(b):
        adjT = np.ascontiguousarray(adj[b, 0].T)
        return _pairs_P_NJ(*_hl8(adjT)), adjT

    def _xp(b):
        xT = np.ascontiguousarray(x[b].T)
        xhi, xlo = _hl8(xT)
        return (_kp_pairs(xhi, L).reshape(P, KP, 2, NC, IC),
                _kp_pairs(xlo, L).reshape(P, KP, 2, NC, IC))

    def _advph(b, g):
        """Heavy head 4+g: int16 A16*(gamma*adj + slope*dist) + B16,
        packed [P, NJ, NC, 2, 256]."""
        gam = float(gamma[0, 4 + g, 0, 0])
        adv = A16 * gam * adjT_by_b[b] + (A16 * slopes[4 + g]) * dist + B16
        adv16 = np.rint(adv).astype(np.int16)
        return np.ascontiguousarray(
            adv16.reshape(NJ, P, NC, 2, 256).transpose(1, 0, 2, 3, 4))

    with ThreadPoolExecutor(max_workers=4) as ex:
        fut_adj = [ex.submit(_adjp, b) for b in range(B)]
        fut_x = [ex.submit(_xp, b) for b in range(B)]
        adj_res = [f.result() for f in fut_adj]
        adjp_by_b = [r[0] for r in adj_res]
        adjT_by_b = [r[1] for r in adj_res]
        x_by_b = [f.result() for f in fut_x]
        fut_adv = {(b, g): ex.submit(_advph, b, g)
                   for b in range(B) for g in range(4)}
        advph_by = {k: f.result() for k, f in fut_adv.items()}

    identT = np.eye(P, dtype=NPBF16)
    eye = np.eye(P, dtype=np.float32)

    in_maps = []
    for core in range(8):
        b, g = divmod(core, 4)
        heads = [8 + 2 * g, 9 + 2 * g, 4 + g, g]  # p0, p1, heavy, light
        qcols = np.concatenate([np.arange(192 * h, 192 * h + 64)
                                for h in heads])
        kcols = qcols + 64
        vcols = qcols + 128
        wqk = WS * weights[:, np.concatenate([qcols, kcols])]
        wqk_h, wqk_l = _hl8(wqk)
        wvm = np.zeros((H, 260), dtype=np.float32)
        for sl in range(4):
            wvm[:, 65 * sl:65 * sl + 64] = \
                weights[:, vcols[64 * sl:64 * sl + 64]]
        wv_h, wv_l = _hl8(WS * wvm)
        vbr = np.zeros((1, 260), dtype=np.float32)
        for sl in range(4):
            vbr[0, 65 * sl:65 * sl + 64] = \
                WS * in_bias[0, 0, vcols[64 * sl:64 * sl + 64]]
            # ones column at 1/8 so denominators come out /8 and the
            # normalized attention lands x8 prescaled for fp8 outproj
            vbr[0, 65 * sl + 64] = WS / 8.0
        owm = np.ascontiguousarray(
            out_w[np.concatenate([np.arange(64 * h, 64 * h + 64)
                                  for h in heads]), :]).astype(NPBF16)
        idgm = np.zeros((P, 2, 512), dtype=NPF8)
        for sl in range(4):
            gi = (gamma[0, heads[sl], 0, 0] * eye).astype(NPF8)
            idgm[:, 0, sl * P:(sl + 1) * P] = gi
            idgm[:, 1, sl * P:(sl + 1) * P] = gi
        idslm = np.zeros((P, 2, 256), dtype=NPF8)
        idslm[:, 0, P:2 * P] = (16.0 * slopes[heads[3]] * eye).astype(NPF8)
        idslm[:, 1, P:2 * P] = (slopes[heads[3]] * eye).astype(NPF8)
        cstm = np.ones((1, IC + 260), dtype=NPBF16)
        cstm[0, IC:] = vbr[0].astype(NPBF16)
        m = {
            "xh": x_by_b[b][0], "xl": x_by_b[b][1],
            "wq": np.ascontiguousarray(np.stack(
                [_kp_pairs(wqk_h, 512), _kp_pairs(wqk_l, 512)], axis=3)),
            "wv": np.ascontiguousarray(np.stack(
                [_kp_pairs(wv_h, 260), _kp_pairs(wv_l, 260)], axis=3)),
            "cst": cstm,
            "adjp": adjp_by_b[b], "distw": distw,
            "advph": advph_by[(b, g)],
            "idg": idgm, "idsl": idslm, "identT": identT,
            "ow": np.ascontiguousarray(
                owm.reshape(2, P, H).transpose(1, 0, 2)),
        }
        if with_qk_bias:
            m["qkb"] = (WS * in_bias[0, 0, np.concatenate([qcols, kcols])]
                        ).reshape(1, -1).astype(NPBF16)
        in_maps.append(m)
    return in_maps


def kernel(x, adj, weights, in_bias, out_w, out_bias, gamma):
    x = np.asarray(x, dtype=np.float32)
    adj = np.asarray(adj, dtype=np.float32)
    weights = np.asarray(weights, dtype=np.float32)
    in_bias = np.asarray(in_bias, dtype=np.float32)
    out_w = np.asarray(out_w, dtype=np.float32)
    out_bias = np.asarray(out_bias, dtype=np.float32)
    gamma = np.asarray(gamma, dtype=np.float32)

    with_qk_bias = bool(np.any(in_bias[0, 0, :]))
    key = f"nc_{with_qk_bias}"
    if key not in _cache:
        _cache[key] = _build_program(with_qk_bias)
    nc = _cache[key]

    in_maps = _build_in_maps(x, adj, weights, in_bias, gamma, out_w,
                             with_qk_bias)
    res = bass_utils.run_bass_kernel_spmd(nc, in_maps, core_ids=list(range(8)),
                                          **RUN_KWARGS)
    _cache["last_result"] = res

    out = np.empty((B, L, H), dtype=np.float32)
    for b in range(B):
        acc = res.results[4 * b]["yT"].astype(np.float32)
        for g in range(1, 4):
            acc += res.results[4 * b + g]["yT"]
        # [P, mp, t, L] -> [H, L]: row m*128+p = acc[p, m//2, m%2]
        yt_full = acc.transpose(1, 2, 0, 3).reshape(H, L)
        out[b] = yt_full.T + out_bias[0, 0][None, :]
    return out

